# revision 5
# baseline (speedup 1.0000x reference)
"""Trainium2 Bass kernel for nn_MoE_4088808866374.

Top-1 MoE (B=4, S=1024, D=1024, E=8, F=2816, K=1) + shared expert.
The reference computes all 8 experts densely over all 4096 tokens, but the
sigmoid gate is exactly 0 for non-top-1 experts (sigmoid(-inf)), and zero
inputs propagate exactly through SwiGLU, so a sparse dispatch computes the
identical result with ~4.5x fewer FLOPs: each token runs 2 SwiGLU passes
(shared + its top-1 expert).

Microbenchmarks on this hardware (micro.py): PE streams fp16 matmuls at
0.263 ns/col (~3.8 GHz effective) and per-core HBM DMA sustains ~435 GB/s.
Per-core work is ~1044 token-passes x 528 cols = 551k cols -> ~145 us of
pure PE streaming, while the fp16 baseline moved ~58 MB -> ~133 us of DMA:
the old schedule rode the DMA roofline and stalled the PE (~63% busy
measured). This version targets a clean PE-bound schedule:

  - Routed-expert w2 in fp8 e3m4 (x64 scale, rescaled in the psum drain):
    w2 is stationary in the y-phase so the matmul still streams at 1
    col/cyc, but its bytes halve. Measured end-to-end rel err ~0.95e-2
    (tolerance 2e-2); shared w2 stays fp16 for margin.
  - y output in bf16 (host combine sums in fp32): -2 MB, adds ~2e-4 err.
  - PE program order h0,h1,y0,h2,y1,y2 with double-buffered mid and w2:
    the ACT/DVE latency tail of each h-phase is hidden behind the next
    segment's h-phase instead of bubbling the PE at every phase switch.
  - mid is double-buffered (bufs=2) so h(i+1) never waits on y(i).
  - y psum drain on DVE (tensor_scalar_mul, applies the 1/64 w2 descale);
    ACT only does silus (Pool cannot access PSUM on TRN2).
  - w13 slabs alternate between the sync and DVE DMA queues; w2 on the
    Pool queue; x/y on the ACT queue: no single queue is asked for more
    than ~180 GB/s even in the smallest segment.

Expert-slot sharding as before: each core runs 3 segments (shared 512
tokens + two expert slots of 292/240 = 1044 token-passes vs the perfectly
balanced 1024). Which expert weights fill each slot is host-side data; a
tiny DFS solver picks slot assignments and capacities auto-grow if counts
shift. Router + dispatch + combine run host-side.
"""

import math

import numpy as np

import concourse.bacc as bacc
import concourse.mybir as mybir
import concourse.tile as tile
from concourse import bass_utils

# Problem constants (hardcoded per harness contract).
B, S, D, E, F = 4, 1024, 1024, 8, 2816
A = B * S            # 4096 tokens
T = A // E           # 512 shared-expert tokens per core
P = 128
D_CH = D // P        # 8
F_CH = F // P        # 22

# (shared, slotA, slotB) tokens per core. 2*292 >= 583 (max expert count for
# the key-0 inputs); the solver in prepare() re-derives/bumps if counts move.
SEGS_DEFAULT = (T, 292, 240)

W2_SCALE = 64.0      # routed w2 quantization scale (fp8 e3m4)

_BUILD_CACHE = {}


def _chunks(n, cap):
    """Near-equal token chunks, each <= cap (PSUM caps a chunk at 512 fp32)."""
    k = math.ceil(n / cap)
    base = n // k
    return [base + (1 if i < n - base * k else 0) for i in range(k)]


def _build(cdt_name: str, segs: tuple, reps: int = 1,
           h_cap: int = 512, y_cap: int = 512, psy_bufs: int = 4,
           w2r_name: str = "float8e3", ydt_name: str = "bfloat16",
           wbufs: int = 8):
    """Build + compile the SPMD Bass kernel for per-core token segments.

    Segment 0 is the shared expert (T tokens, w2 in cdt), segments 1.. are
    expert slots (w2 in w2r_name, scaled by W2_SCALE) whose weight contents
    are chosen host-side.

    reps>1 wraps the body in a hardware For_i loop (used by the test harness
    to measure per-execution device time as a slope, amortizing the ~100ms
    axon dispatch overhead)."""
    key = (cdt_name, tuple(segs), reps, h_cap, y_cap, psy_bufs, w2r_name,
           ydt_name, wbufs)
    if key in _BUILD_CACHE:
        return _BUILD_CACHE[key]

    sdt = getattr(mybir.dt, cdt_name)
    ydt = getattr(mybir.dt, ydt_name)
    fp32 = mybir.dt.float32
    nseg = len(segs)

    nc = bacc.Bacc("TRN2", target_bir_lowering=False, debug=False)

    # DRAM I/O (per core). Host-packed layouts, contiguous per partition:
    #   x{i}:   [P(d_inner), D_CH, n]
    #   w13{i}: [P(d_inner), F_CH, 2, D_CH, P(f_inner)]  (w1|w3 per f-chunk)
    #   w2{i}:  [P(f_inner), F_CH, D]   seg 0: cdt; segs 1..: w2r * W2_SCALE
    #   y{i}:   [D, n] ydt (y transposed: partition dim = d)
    xs_d, w13_d, w2_d, ys_d = [], [], [], []
    for i, n in enumerate(segs):
        w2dt = sdt if i == 0 else getattr(mybir.dt, w2r_name)
        xs_d.append(nc.dram_tensor(f"x{i}", [P, D_CH, n], sdt,
                                   kind="ExternalInput"))
        w13_d.append(nc.dram_tensor(f"w13_{i}", [P, F_CH, 2, D_CH, P], sdt,
                                    kind="ExternalInput"))
        w2_d.append(nc.dram_tensor(f"w2_{i}", [P, F_CH, D], w2dt,
                                   kind="ExternalInput"))
        ys_d.append(nc.dram_tensor(f"y{i}", [D, n], ydt,
                                   kind="ExternalOutput"))
    # tiny pass-through token so the test harness can chain executions
    tok = nc.dram_tensor("tok", [1, 1], fp32, kind="ExternalInput")
    tokout = nc.dram_tensor("tokout", [1, 1], fp32, kind="ExternalOutput")

    with tile.TileContext(nc) as tc:
        with tc.tile_pool(name="xpool", bufs=1) as xpool, \
             tc.tile_pool(name="wpool", bufs=wbufs) as wpool, \
             tc.tile_pool(name="w2s", bufs=1) as w2spool, \
             tc.tile_pool(name="w2r", bufs=2) as w2rpool, \
             tc.tile_pool(name="midpool", bufs=2) as midpool, \
             tc.tile_pool(name="tmp", bufs=2) as tmp, \
             tc.tile_pool(name="ytmp", bufs=3) as ytmp, \
             tc.tile_pool(name="psA", bufs=2, space="PSUM") as psA, \
             tc.tile_pool(name="psB", bufs=2, space="PSUM") as psB, \
             tc.tile_pool(name="psY", bufs=psy_bufs, space="PSUM") as psY:

            x_sb = [None] * nseg
            w2_sb = [None] * nseg
            mid_sb = [None] * nseg

            def load_x(i):
                n = segs[i]
                x_sb[i] = xpool.tile([P, D_CH, n], sdt, tag=f"x{i}",
                                     name=f"x_{i}")
                for d in range(D_CH):
                    nc.scalar.dma_start(x_sb[i][:, d], xs_d[i].ap()[:, d])

            def h_phase(i):
                n = segs[i]
                w2dt = sdt if i == 0 else getattr(mybir.dt, w2r_name)
                pool = w2spool if i == 0 else w2rpool
                w2_sb[i] = pool.tile([P, F_CH, D], w2dt,
                                     tag="w2s" if i == 0 else "w2r",
                                     name=f"w2_{i}")
                mid_sb[i] = midpool.tile([P, F_CH, n], sdt, tag="mid",
                                         name=f"mid_{i}")
                hch = _chunks(n, h_cap)
                for fc in range(F_CH):
                    w_sb = wpool.tile([P, 2, D_CH, P], sdt, tag="w13slab",
                                      name=f"w13s_{i}_{fc}")
                    weng = nc.sync if fc % 2 == 0 else nc.scalar
                    weng.dma_start(w_sb[:], w13_d[i].ap()[:, fc])
                    nc.gpsimd.dma_start(w2_sb[i][:, fc], w2_d[i].ap()[:, fc])
                    t0 = 0
                    for tn in hch:
                        ps1 = psA.tile([P, 512], fp32, tag="ps1",
                                       name=f"ps1_{i}_{fc}_{t0}")[:, :tn]
                        for d in range(D_CH):
                            nc.tensor.matmul(
                                ps1, w_sb[:, 0, d],
                                x_sb[i][:, d, t0:t0 + tn],
                                start=(d == 0), stop=(d == D_CH - 1))
                        ps3 = psB.tile([P, 512], fp32, tag="ps3",
                                       name=f"ps3_{i}_{fc}_{t0}")[:, :tn]
                        for d in range(D_CH):
                            nc.tensor.matmul(
                                ps3, w_sb[:, 1, d],
                                x_sb[i][:, d, t0:t0 + tn],
                                start=(d == 0), stop=(d == D_CH - 1))
                        silu_sb = tmp.tile([P, 512], fp32, tag="silu",
                                           name=f"silu_{i}_{fc}_{t0}")[:, :tn]
                        nc.scalar.activation(silu_sb, ps1,
                                             mybir.ActivationFunctionType.Silu)
                        nc.vector.tensor_tensor(mid_sb[i][:, fc, t0:t0 + tn],
                                                silu_sb, ps3,
                                                mybir.AluOpType.mult)
                        t0 += tn

            def y_phase(i):
                # y[d, t] = sum_f w2[f, d] * mid[f, t]; stationary = w2 block
                # [128f x 128d], moving = mid tokens. Pool drains psum with
                # the w2 descale fused in.
                n = segs[i]
                descale = 1.0 if i == 0 else 1.0 / W2_SCALE
                t0 = 0
                for tn in _chunks(n, y_cap):
                    for dt in range(D_CH):
                        psy = psY.tile([P, min(512, y_cap)], fp32, tag="psy",
                                       name=f"psy_{i}_{t0}_{dt}")[:, :tn]
                        for fc in range(F_CH):
                            nc.tensor.matmul(
                                psy,
                                w2_sb[i][:, fc, dt * P:(dt + 1) * P],
                                mid_sb[i][:, fc, t0:t0 + tn],
                                start=(fc == 0), stop=(fc == F_CH - 1))
                        y_sb = ytmp.tile([P, min(512, y_cap)], ydt, tag="ysb",
                                         name=f"y_{i}_{t0}_{dt}")[:, :tn]
                        nc.vector.tensor_scalar_mul(y_sb, psy, descale)
                        nc.scalar.dma_start(
                            ys_d[i].ap()[dt * P:(dt + 1) * P, t0:t0 + tn],
                            y_sb)
                    t0 += tn

            def body():
                # h0, h1, y0, h2, y1, ..., y(n-1): each y(i) is emitted after
                # h(i+1) so the PE never waits on the mid latency tail.
                for i in range(nseg):
                    load_x(i)
                h_phase(0)
                for i in range(1, nseg):
                    h_phase(i)
                    y_phase(i - 1)
                y_phase(nseg - 1)

            if reps == 1:
                body()
            else:
                # staggered_reset avoids the ~2us all-engine barrier per
                # back-edge; hint PE so the >256-inst body's back-edge
                # branch target is prefetched into IRAM
                with tc.For_i(0, reps, 1, staggered_reset=True,
                              hint_engines=(mybir.EngineType.PE,)):
                    body()
            nc.sync.dma_start(tokout.ap(), tok.ap())

    nc.compile()
    _BUILD_CACHE[key] = nc
    return nc


def _sigmoid32(x):
    x = x.astype(np.float32)
    return np.where(x >= 0, 1.0 / (1.0 + np.exp(-x)),
                    np.exp(x) / (1.0 + np.exp(x))).astype(np.float32)


def _np_dt(cdt_name):
    import ml_dtypes
    if cdt_name == "bfloat16":
        return ml_dtypes.bfloat16
    if cdt_name == "float16":
        return np.float16
    if cdt_name == "float8e3":
        return ml_dtypes.float8_e3m4
    return np.float32


def _pack_w13(w1, w3, np_dt):
    # 2x [D, F] -> [P(d_inner), F_CH, 2, D_CH, P(f_inner)]
    def pk(w):
        return w.reshape(D_CH, P, F_CH, P).transpose(1, 2, 0, 3)
    return np.ascontiguousarray(
        np.stack([pk(w1), pk(w3)], axis=2).astype(np_dt))


def _pack_w2(w, np_dt, scale=1.0):
    # [F, D] -> [P(f_inner), F_CH, D]
    return np.ascontiguousarray(
        (w.reshape(F_CH, P, D).transpose(1, 0, 2) * scale).astype(np_dt))


def _pack_xT(x, np_dt, n):
    # [k, D] (k<=n, zero-padded to n) -> [P(d_inner), D_CH, n]
    if x.shape[0] < n:
        x = np.concatenate(
            [x, np.zeros((n - x.shape[0], D), np.float32)], axis=0)
    return np.ascontiguousarray(
        x.reshape(n, D_CH, P).transpose(2, 1, 0).astype(np_dt))


def _solve_slots(counts, caps):
    """Assign each expert a vector a[j] of slots per size-class j (8 slots
    per class, class j holds caps[j] tokens) with sum_j a[j]*caps[j] >=
    count and per-class totals <= E. Returns [a_e vectors] or None."""
    k = len(caps)

    def options(n):
        # pareto set of slot-count vectors covering n tokens
        opts = set()

        def rec(j, vec, cov):
            if cov >= n:
                opts.add(tuple(vec) + (0,) * (k - len(vec)))
                return
            if j == k:
                return
            maxa = min(E, -(-(n - cov) // caps[j]))
            for a in range(maxa + 1):
                vec.append(a)
                rec(j + 1, vec, cov + a * caps[j])
                vec.pop()
        rec(0, [], 0)
        # prune dominated (elementwise >=)
        out = []
        for v in sorted(opts, key=sum):
            if not any(all(u[i] <= v[i] for i in range(k)) and u != v
                       for u in out):
                out.append(v)
        return out

    opts = [options(int(n)) for n in counts]
    order = np.argsort(counts)[::-1]
    assign = [None] * len(counts)

    def dfs(i, used):
        if i == len(order):
            return True
        e = order[i]
        for v in opts[e]:
            nu = tuple(used[j] + v[j] for j in range(k))
            if all(u <= E for u in nu):
                assign[e] = v
                if dfs(i + 1, nu):
                    return True
        assign[e] = None
        return False

    return assign if dfs(0, (0,) * k) else None


def prepare(x_bsD, router_DE, w1_eDF, w3_eDF, w2_eFD, ws1_DF, ws3_DF, ws2_FD,
            cdt_name="float16", segs=SEGS_DEFAULT, w2r_name="float8e3"):
    """Host-side routing + dispatch. Returns (in_maps, aux) for the SPMD run."""
    np_dt = _np_dt(cdt_name)
    w2r_dt = _np_dt(w2r_name)

    x = np.ascontiguousarray(np.asarray(x_bsD, np.float32).reshape(A, D))
    scores = x @ np.asarray(router_DE, np.float32)          # [A, E]
    top1 = np.argmax(scores, axis=1)                        # [A]
    gate = _sigmoid32(scores[np.arange(A), top1])           # [A]

    idx_e = [np.nonzero(top1 == e)[0] for e in range(E)]
    counts = np.array([len(i) for i in idx_e])

    caps = list(segs[1:])
    assign = _solve_slots(counts, caps)
    while assign is None:
        caps = [c + 16 for c in caps]
        assign = _solve_slots(counts, caps)
    segs = (T, *caps)

    # Distribute each expert's tokens into its slots (largest class first).
    slots = [[] for _ in caps]   # per class: list of (expert, token_idx)
    for e in range(E):
        pos = 0
        for j in range(len(caps)):
            for _ in range(assign[e][j]):
                take = min(caps[j], counts[e] - pos)
                slots[j].append((e, idx_e[e][pos:pos + take]))
                pos += take
        assert pos >= counts[e]
    for j in range(len(caps)):
        while len(slots[j]) < E:
            slots[j].append((0, np.zeros(0, np.int64)))

    w13p = {}
    w2p = {}
    for e in range(E):
        w13p[e] = _pack_w13(np.asarray(w1_eDF[e], np.float32),
                            np.asarray(w3_eDF[e], np.float32), np_dt)
        w2p[e] = _pack_w2(np.asarray(w2_eFD[e], np.float32), w2r_dt,
                          scale=W2_SCALE)
    ws13p = _pack_w13(np.asarray(ws1_DF, np.float32),
                      np.asarray(ws3_DF, np.float32), np_dt)
    ws2p = _pack_w2(np.asarray(ws2_FD, np.float32), np_dt)

    in_maps = []
    for c in range(E):
        m = {
            "x0": _pack_xT(x[c * T:(c + 1) * T], np_dt, T),
            "w13_0": ws13p, "w2_0": ws2p,
            "tok": np.zeros((1, 1), np.float32),
        }
        for j in range(len(caps)):
            e, idx = slots[j][c]
            m[f"x{j + 1}"] = _pack_xT(gate[idx, None] * x[idx], np_dt,
                                      caps[j])
            m[f"w13_{j + 1}"] = w13p[e]
            m[f"w2_{j + 1}"] = w2p[e]
        in_maps.append(m)
    return in_maps, (slots, segs)


def combine(results, aux):
    """Merge per-core outputs into the full [B, S, D] output."""
    slots, segs = aux
    out = np.empty((A, D), np.float32)
    for c in range(E):
        out[c * T:(c + 1) * T] = results[c]["y0"].astype(np.float32).T
    for c in range(E):
        for j in range(len(segs) - 1):
            _, idx = slots[j][c]
            if len(idx):
                out[idx] += results[c][f"y{j + 1}"][:, :len(idx)] \
                    .astype(np.float32).T
    return out.reshape(B, S, D)


def kernel(x_bsD, router_DE, w1_eDF, w3_eDF, w2_eFD, ws1_DF, ws3_DF, ws2_FD,
           cdt_name="float16", segs=SEGS_DEFAULT):
    in_maps, aux = prepare(x_bsD, router_DE, w1_eDF, w3_eDF, w2_eFD,
                           ws1_DF, ws3_DF, ws2_FD, cdt_name=cdt_name,
                           segs=segs)
    nc = _build(cdt_name, aux[1])
    res = bass_utils.run_bass_kernel_spmd(nc, in_maps, core_ids=list(range(E)))
    return combine(res.results, aux)
